# revision 15
# baseline (speedup 1.0000x reference)
"""Expert-parallel grouped-MLP (MoE experts) kernel for 8 Trainium2 cores.

Problem: y = W2_e @ silu(W1_e @ x_e + b1_e) + b2_e for E=16 independent
experts (grouped 1x1 conv), B=8 batches, C=256 channels/expert, CAP=4,
L=1024 positions.

Sharding: expert-parallel — core i owns experts {2i, 2i+1}; no cross-core
communication. Host pre-transposes weights into lhsT layout; each core runs
an fp16 matmul pipeline (DVE casts fp32->fp16 on the fly):

  per (b, e) pair:  DMA x[256,1024] -> 2 k-tiles
    layer 1: 8 m-tiles x (2k x 2n) matmuls -> PSUM[128,1024]
             ACT silu(. + b1) PSUM -> h SBUF [128, 8x1024] (fp32r)
    layer 2: 2 j-tiles x (8q x 2n) accumulating matmuls -> PSUM[128,1024]
             DVE + b2 PSUM -> y SBUF, DMA out

Startup: ~20 zero-weight bf16 warmup matmuls keep the PE busy (and HAM
warm) while the first weights/x stream in; DMA issue order puts pair-0
data first.
"""
import numpy as np

import concourse.bass as bass
import concourse.tile as tile
from concourse import bacc, mybir
from concourse.bass_utils import run_bass_kernel_spmd

# Problem constants (hardcoded per contract)
B, E, C, CAP, L = 8, 16, 256, 4, 1024
F = C * CAP            # 1024 hidden per expert
NCORES = 8
EPC = E // NCORES      # 2 experts per core
P = 128                # partitions
KT = C // P            # 2 k-tiles (layer-1 contraction)
MT = F // P            # 8 m-tiles (layer-1 output partitions)
JT = C // P            # 2 j-tiles (layer-2 output partitions)
QT = F // P            # 8 q-tiles (layer-2 contraction)
NT = L // 512          # 2 n-tiles of 512 cols
N_WARMUP = 16          # dummy PE warmup matmuls

_FP32 = mybir.dt.float32
_FP32R = mybir.dt.float32r
_FP16 = mybir.dt.float16


def _build():
    nc = bacc.Bacc("TRN2", target_bir_lowering=False, debug=False)

    xs_d = nc.dram_tensor("xs", [B, EPC * C, L], _FP32, kind="ExternalInput")
    w1t_d = nc.dram_tensor("w1t", [EPC, C, F], _FP32, kind="ExternalInput")
    b1s_d = nc.dram_tensor("b1s", [EPC, F], _FP32, kind="ExternalInput")
    w2t_d = nc.dram_tensor("w2t", [EPC, F, C], _FP32, kind="ExternalInput")
    b2s_d = nc.dram_tensor("b2s", [EPC, C], _FP32, kind="ExternalInput")
    ys_d = nc.dram_tensor("ys", [B, EPC * C, L], _FP32, kind="ExternalOutput")

    with tile.TileContext(nc) as tc:
        with (
            tc.tile_pool(name="const", bufs=1) as cpool,
            tc.tile_pool(name="x", bufs=8) as xpool,
            tc.tile_pool(name="h", bufs=3) as hpool,
            tc.tile_pool(name="y", bufs=3) as ypool,
            tc.tile_pool(name="ps", bufs=4, space="PSUM") as pspool,
        ):
            # ---- PE warmup: zero bf16 matmuls with no DMA deps ----
            wdum = cpool.tile([P, P], mybir.dt.bfloat16, tag="wdum")
            rdum = cpool.tile([P, 512], mybir.dt.bfloat16, tag="rdum")
            nc.vector.memset(wdum[:], 0.0)
            nc.vector.memset(rdum[:], 0.0)
            for i in range(N_WARMUP):
                pdum = pspool.tile([P, L], _FP32, tag="ps")
                nc.tensor.matmul(pdum[:, :512], wdum[:], rdum[:],
                                 start=True, stop=True)

            # ---- weight/bias tiles (declared; DMAs ordered for startup) ----
            # w1sb[e][k]: [128, F];   [p, f] = W1T[e, k*128+p, f]
            # w2sb[e]:    [128, QT*C]; [p, q*C+c] = W2T[e, q*128+p, c]
            w1sb = [[cpool.tile([P, F], _FP16, tag=f"w1_{e}_{k}",
                               name=f"w1sb_{e}_{k}")
                     for k in range(KT)] for e in range(EPC)]
            w2sb = [cpool.tile([P, QT * C], _FP16, tag=f"w2_{e}",
                               name=f"w2sb_{e}")
                    for e in range(EPC)]
            w1st = [[cpool.tile([P, F], _FP32, tag=f"w1s_{e}_{k}",
                                name=f"w1st_{e}_{k}")
                     for k in range(KT)] for e in range(EPC)]
            w2st = [cpool.tile([P, QT * C], _FP32, tag=f"w2s_{e}",
                               name=f"w2st_{e}")
                    for e in range(EPC)]
            b1sb = cpool.tile([P, EPC * MT], _FP32, tag="b1")  # col e*MT+m
            b2sb = cpool.tile([P, EPC * JT], _FP32, tag="b2")  # col e*JT+j

            def load_w1(e, k):
                nc.sync.dma_start(
                    w1st[e][k][:],
                    w1t_d.ap()[e, k * P:(k + 1) * P, :],
                )
                nc.vector.tensor_copy(w1sb[e][k][:], w1st[e][k][:])

            def load_w2(e, q):
                nc.sync.dma_start(
                    w2st[e][:, q * C:(q + 1) * C],
                    w2t_d.ap()[e, q * P:(q + 1) * P, :],
                )

            def cast_w2(e):
                nc.vector.tensor_copy(w2sb[e][:], w2st[e][:])

            def load_b(e):
                nc.sync.dma_start(
                    b1sb[:, e * MT:(e + 1) * MT],
                    b1s_d.ap()[e].rearrange("(m p) -> p m", p=P),
                )
                nc.sync.dma_start(
                    b2sb[:, e * JT:(e + 1) * JT],
                    b2s_d.ap()[e].rearrange("(j p) -> p j", p=P),
                )

            def load_x(b, e):
                # two k-tiles [128, L], each a contiguous 512 KB DRAM slab,
                # staged fp32 then DVE-cast to fp16
                tiles = []
                for k in range(KT):
                    xst = xpool.tile([P, L], _FP32, tag="xst", name=f"xst_{b}_{e}_{k}")
                    nc.sync.dma_start(
                        xst[:],
                        xs_d.ap()[b, e * C + k * P: e * C + (k + 1) * P, :],
                    )
                    xt = xpool.tile([P, L], _FP16, tag="x", name=f"x_{b}_{e}_{k}")
                    nc.vector.tensor_copy(xt[:], xst[:])
                    tiles.append(xt)
                return tiles

            def load_x_split(b, e):
                # first pair: per-n half tiles so MM 0 waits on 256 KB only
                tiles = []
                for k in range(KT):
                    halves = []
                    for n in range(NT):
                        xst = xpool.tile([P, 512], _FP32, tag="xst0",
                                         name=f"xst0_{k}_{n}")
                        nc.sync.dma_start(
                            xst[:],
                            xs_d.ap()[b, e * C + k * P: e * C + (k + 1) * P,
                                      n * 512:(n + 1) * 512],
                        )
                        xt = xpool.tile([P, 512], _FP16, tag="x0",
                                        name=f"x0_{k}_{n}")
                        nc.vector.tensor_copy(xt[:], xst[:])
                        halves.append(xt)
                    tiles.append(halves)
                return tiles

            # startup-critical order: pair-0 inputs first; expert-1 weights
            # deferred until after early x prefetches (needed only at pair 8)
            load_w1(0, 0)
            x0 = load_x_split(0, 0)
            load_w1(0, 1)
            load_b(0)
            for q in range(QT):
                load_w2(0, q)
            cast_w2(0)

            # ---- per-(expert, batch) pipeline ----
            for e in range(EPC):
                for b in range(B):
                    xsb = x0 if (e == 0 and b == 0) else load_x(b, e)
                    if e == 0 and b == 1:
                        # early x prefetches are in flight; now queue
                        # expert-1 weights (needed at pair 8, ~130us in)
                        load_w1(1, 0)
                        load_w1(1, 1)
                        load_b(1)
                        for q in range(QT):
                            load_w2(1, q)
                        cast_w2(1)

                    # layer 1: h = silu(W1 @ x + b1), h[p, m*L + l]
                    hsb = hpool.tile([P, MT * L], _FP16, tag="h")
                    for m in range(MT):
                        psh = pspool.tile([P, L], _FP32, tag="ps")
                        for k in range(KT):
                            for n in range(NT):
                                rhs = (xsb[k][n][:] if (e == 0 and b == 0)
                                       else xsb[k][:, n * 512:(n + 1) * 512])
                                nc.tensor.matmul(
                                    psh[:, n * 512:(n + 1) * 512],
                                    w1sb[e][k][:, m * P:(m + 1) * P],
                                    rhs,
                                    start=(k == 0),
                                    stop=(k == KT - 1),
                                )
                        nc.scalar.activation(
                            hsb[:, m * L:(m + 1) * L],
                            psh[:],
                            mybir.ActivationFunctionType.Silu,
                            bias=b1sb[:, e * MT + m: e * MT + m + 1],
                        )

                    # layer 2: y = W2 @ h + b2
                    for j in range(JT):
                        psy = pspool.tile([P, L], _FP32, tag="ps")
                        for q in range(QT):
                            for n in range(NT):
                                nc.tensor.matmul(
                                    psy[:, n * 512:(n + 1) * 512],
                                    w2sb[e][:, q * C + j * P: q * C + (j + 1) * P],
                                    hsb[:, q * L + n * 512: q * L + (n + 1) * 512],
                                    start=(q == 0),
                                    stop=(q == QT - 1),
                                )
                        ysb = ypool.tile([P, L], _FP32, tag="y",
                                         name=f"ysb_{e}_{b}_{j}")
                        nc.vector.tensor_scalar_add(
                            ysb[:],
                            psy[:],
                            b2sb[:, e * JT + j: e * JT + j + 1],
                        )
                        nc.sync.dma_start(
                            ys_d.ap()[b, e * C + j * P: e * C + (j + 1) * P, :],
                            ysb[:],
                        )

    nc.compile()
    return nc


_NC_CACHE = None


def _get_nc():
    global _NC_CACHE
    if _NC_CACHE is None:
        _NC_CACHE = _build()
    return _NC_CACHE


def _shard_inputs(x, W1, b1, W2, b2):
    """Full inputs -> list of 8 per-core input dicts (expert-parallel)."""
    x = np.ascontiguousarray(x, dtype=np.float32)
    # lhsT layouts: W1T[e] = W1r[e].T -> [E, C, F]; W2T[e] = W2r[e].T -> [E, F, C]
    w1t = np.ascontiguousarray(
        W1.astype(np.float32).reshape(E, F, C).transpose(0, 2, 1)
    )
    w2t = np.ascontiguousarray(
        W2.astype(np.float32).reshape(E, C, F).transpose(0, 2, 1)
    )
    b1r = np.ascontiguousarray(b1.astype(np.float32).reshape(E, F))
    b2r = np.ascontiguousarray(b2.astype(np.float32).reshape(E, C))
    in_maps = []
    for i in range(NCORES):
        es = slice(i * EPC, (i + 1) * EPC)
        in_maps.append({
            "xs": np.ascontiguousarray(x[:, i * EPC * C:(i + 1) * EPC * C, :]),
            "w1t": np.ascontiguousarray(w1t[es]),
            "b1s": np.ascontiguousarray(b1r[es]),
            "w2t": np.ascontiguousarray(w2t[es]),
            "b2s": np.ascontiguousarray(b2r[es]),
        })
    return in_maps


def run(x, W1, b1, W2, b2, trace=False, **trace_kwargs):
    nc = _get_nc()
    in_maps = _shard_inputs(x, W1, b1, W2, b2)
    res = run_bass_kernel_spmd(
        nc, in_maps, core_ids=list(range(NCORES)), trace=trace, **trace_kwargs
    )
    y = np.concatenate([res.results[i]["ys"] for i in range(NCORES)], axis=1)
    return y, res


def kernel(x, W1, b1, W2, b2):
    y, _ = run(x, W1, b1, W2, b2)
    return y.astype(np.float32)


# revision 16
# speedup vs baseline: 1.0311x; 1.0311x over previous
"""Expert-parallel grouped-MLP (MoE experts) kernel for 8 Trainium2 cores.

Problem: y = W2_e @ silu(W1_e @ x_e + b1_e) + b2_e for E=16 independent
experts (grouped 1x1 conv), B=8 batches, C=256 channels/expert, CAP=4,
L=1024 positions.

Sharding: expert-parallel — core i owns experts {2i, 2i+1}; no cross-core
communication. Host pre-transposes weights into lhsT layout; each core runs
an fp16 matmul pipeline (DVE casts fp32->fp16 on the fly):

  per (b, e) pair:  DMA x[256,1024] -> 2 k-tiles
    layer 1: 8 m-tiles x (2k x 2n) matmuls -> PSUM[128,1024]
             ACT silu(. + b1) PSUM -> h SBUF [128, 8x1024] (fp32r)
    layer 2: 2 j-tiles x (8q x 2n) accumulating matmuls -> PSUM[128,1024]
             DVE + b2 PSUM -> y SBUF, DMA out

Startup: ~20 zero-weight bf16 warmup matmuls keep the PE busy (and HAM
warm) while the first weights/x stream in; DMA issue order puts pair-0
data first.
"""
import numpy as np

import concourse.bass as bass
import concourse.tile as tile
from concourse import bacc, mybir
from concourse.bass_utils import run_bass_kernel_spmd

# Problem constants (hardcoded per contract)
B, E, C, CAP, L = 8, 16, 256, 4, 1024
F = C * CAP            # 1024 hidden per expert
NCORES = 8
EPC = E // NCORES      # 2 experts per core
P = 128                # partitions
KT = C // P            # 2 k-tiles (layer-1 contraction)
MT = F // P            # 8 m-tiles (layer-1 output partitions)
JT = C // P            # 2 j-tiles (layer-2 output partitions)
QT = F // P            # 8 q-tiles (layer-2 contraction)
NT = L // 512          # 2 n-tiles of 512 cols
N_WARMUP = 16          # dummy PE warmup matmuls

_FP32 = mybir.dt.float32
_FP32R = mybir.dt.float32r
_FP16 = mybir.dt.float16


def _build():
    nc = bacc.Bacc("TRN2", target_bir_lowering=False, debug=False)

    xs_d = nc.dram_tensor("xs", [B, EPC * C, L], _FP32, kind="ExternalInput")
    w1t_d = nc.dram_tensor("w1t", [EPC, C, F], _FP32, kind="ExternalInput")
    b1s_d = nc.dram_tensor("b1s", [EPC, F], _FP32, kind="ExternalInput")
    w2t_d = nc.dram_tensor("w2t", [EPC, F, C], _FP32, kind="ExternalInput")
    b2s_d = nc.dram_tensor("b2s", [EPC, C], _FP32, kind="ExternalInput")
    ys_d = nc.dram_tensor("ys", [B, EPC * C, L], _FP32, kind="ExternalOutput")

    with tile.TileContext(nc) as tc:
        with (
            tc.tile_pool(name="const", bufs=1) as cpool,
            tc.tile_pool(name="x", bufs=8) as xpool,
            tc.tile_pool(name="h", bufs=2) as hpool,
            tc.tile_pool(name="y", bufs=3) as ypool,
            tc.tile_pool(name="ps", bufs=4, space="PSUM") as pspool,
        ):
            # ---- PE warmup: zero bf16 matmuls with no DMA deps ----
            wdum = cpool.tile([P, P], mybir.dt.bfloat16, tag="wdum")
            rdum = cpool.tile([P, 512], mybir.dt.bfloat16, tag="rdum")
            nc.vector.memset(wdum[:], 0.0)
            nc.vector.memset(rdum[:], 0.0)
            for i in range(N_WARMUP):
                pdum = pspool.tile([P, L], _FP32, tag="ps")
                nc.tensor.matmul(pdum[:, :512], wdum[:], rdum[:],
                                 start=True, stop=True)

            # ---- weight/bias tiles (declared; DMAs ordered for startup) ----
            # w1sb[e][k]: [128, F];   [p, f] = W1T[e, k*128+p, f]
            # w2sb[e]:    [128, QT*C]; [p, q*C+c] = W2T[e, q*128+p, c]
            w1sb = [[cpool.tile([P, F], _FP16, tag=f"w1_{e}_{k}",
                               name=f"w1sb_{e}_{k}")
                     for k in range(KT)] for e in range(EPC)]
            w2sb = [cpool.tile([P, QT * C], _FP16, tag=f"w2_{e}",
                               name=f"w2sb_{e}")
                    for e in range(EPC)]
            w1st = [[cpool.tile([P, F], _FP32, tag=f"w1s_{e}_{k}",
                                name=f"w1st_{e}_{k}")
                     for k in range(KT)] for e in range(EPC)]
            w2st = [cpool.tile([P, QT * C], _FP32, tag=f"w2s_{e}",
                               name=f"w2st_{e}")
                    for e in range(EPC)]
            b1sb = cpool.tile([P, EPC * MT], _FP32, tag="b1")  # col e*MT+m
            b2sb = cpool.tile([P, EPC * JT], _FP32, tag="b2")  # col e*JT+j

            def load_w1(e, k):
                nc.sync.dma_start(
                    w1st[e][k][:],
                    w1t_d.ap()[e, k * P:(k + 1) * P, :],
                )
                nc.vector.tensor_copy(w1sb[e][k][:], w1st[e][k][:])

            def load_w2(e, q):
                nc.sync.dma_start(
                    w2st[e][:, q * C:(q + 1) * C],
                    w2t_d.ap()[e, q * P:(q + 1) * P, :],
                )

            def cast_w2(e):
                nc.vector.tensor_copy(w2sb[e][:], w2st[e][:])

            def load_b(e):
                nc.sync.dma_start(
                    b1sb[:, e * MT:(e + 1) * MT],
                    b1s_d.ap()[e].rearrange("(m p) -> p m", p=P),
                )
                nc.sync.dma_start(
                    b2sb[:, e * JT:(e + 1) * JT],
                    b2s_d.ap()[e].rearrange("(j p) -> p j", p=P),
                )

            def load_x(b, e):
                # two k-tiles [128, L], each a contiguous 512 KB DRAM slab,
                # staged fp32 then DVE-cast to fp16
                tiles = []
                for k in range(KT):
                    xst = xpool.tile([P, L], _FP32, tag="xst", name=f"xst_{b}_{e}_{k}")
                    nc.sync.dma_start(
                        xst[:],
                        xs_d.ap()[b, e * C + k * P: e * C + (k + 1) * P, :],
                    )
                    xt = xpool.tile([P, L], _FP16, tag="x", name=f"x_{b}_{e}_{k}")
                    nc.vector.tensor_copy(xt[:], xst[:])
                    tiles.append(xt)
                return tiles

            def load_x_split(b, e):
                # first pair: per-n half tiles so MM 0 waits on 256 KB only
                tiles = []
                for k in range(KT):
                    halves = []
                    for n in range(NT):
                        xst = xpool.tile([P, 512], _FP32, tag="xst0",
                                         name=f"xst0_{k}_{n}")
                        nc.sync.dma_start(
                            xst[:],
                            xs_d.ap()[b, e * C + k * P: e * C + (k + 1) * P,
                                      n * 512:(n + 1) * 512],
                        )
                        xt = xpool.tile([P, 512], _FP16, tag="x0",
                                        name=f"x0_{k}_{n}")
                        nc.vector.tensor_copy(xt[:], xst[:])
                        halves.append(xt)
                    tiles.append(halves)
                return tiles

            # startup-critical order: pair-0 inputs first; expert-1 weights
            # deferred until after early x prefetches (needed only at pair 8)
            load_w1(0, 0)
            x0 = load_x_split(0, 0)
            load_w1(0, 1)
            load_b(0)
            for q in range(QT):
                load_w2(0, q)
            cast_w2(0)

            # ---- per-(expert, batch) pipeline ----
            for e in range(EPC):
                for b in range(B):
                    xsb = x0 if (e == 0 and b == 0) else load_x(b, e)
                    if e == 0 and b == 1:
                        # early x prefetches are in flight; now queue
                        # expert-1 weights (needed at pair 8, ~130us in)
                        load_w1(1, 0)
                        load_w1(1, 1)
                        load_b(1)
                        for q in range(QT):
                            load_w2(1, q)
                        cast_w2(1)

                    # layer 1: h = silu(W1 @ x + b1), h[p, m*L + l]
                    hsb = hpool.tile([P, MT * L], _FP16, tag="h")
                    for m in range(MT):
                        psh = pspool.tile([P, L], _FP32, tag="ps")
                        for k in range(KT):
                            for n in range(NT):
                                rhs = (xsb[k][n][:] if (e == 0 and b == 0)
                                       else xsb[k][:, n * 512:(n + 1) * 512])
                                nc.tensor.matmul(
                                    psh[:, n * 512:(n + 1) * 512],
                                    w1sb[e][k][:, m * P:(m + 1) * P],
                                    rhs,
                                    start=(k == 0),
                                    stop=(k == KT - 1),
                                )
                        nc.scalar.activation(
                            hsb[:, m * L:(m + 1) * L],
                            psh[:],
                            mybir.ActivationFunctionType.Silu,
                            bias=b1sb[:, e * MT + m: e * MT + m + 1],
                        )

                    # layer 2: y = W2 @ h + b2
                    for j in range(JT):
                        psy = pspool.tile([P, L], _FP32, tag="ps")
                        for q in range(QT):
                            for n in range(NT):
                                nc.tensor.matmul(
                                    psy[:, n * 512:(n + 1) * 512],
                                    w2sb[e][:, q * C + j * P: q * C + (j + 1) * P],
                                    hsb[:, q * L + n * 512: q * L + (n + 1) * 512],
                                    start=(q == 0),
                                    stop=(q == QT - 1),
                                )
                        ysb = ypool.tile([P, L], _FP32, tag="y",
                                         name=f"ysb_{e}_{b}_{j}")
                        nc.vector.tensor_scalar_add(
                            ysb[:],
                            psy[:],
                            b2sb[:, e * JT + j: e * JT + j + 1],
                        )
                        nc.sync.dma_start(
                            ys_d.ap()[b, e * C + j * P: e * C + (j + 1) * P, :],
                            ysb[:],
                        )

    nc.compile()
    return nc


_NC_CACHE = None


def _get_nc():
    global _NC_CACHE
    if _NC_CACHE is None:
        _NC_CACHE = _build()
    return _NC_CACHE


def _shard_inputs(x, W1, b1, W2, b2):
    """Full inputs -> list of 8 per-core input dicts (expert-parallel)."""
    x = np.ascontiguousarray(x, dtype=np.float32)
    # lhsT layouts: W1T[e] = W1r[e].T -> [E, C, F]; W2T[e] = W2r[e].T -> [E, F, C]
    w1t = np.ascontiguousarray(
        W1.astype(np.float32).reshape(E, F, C).transpose(0, 2, 1)
    )
    w2t = np.ascontiguousarray(
        W2.astype(np.float32).reshape(E, C, F).transpose(0, 2, 1)
    )
    b1r = np.ascontiguousarray(b1.astype(np.float32).reshape(E, F))
    b2r = np.ascontiguousarray(b2.astype(np.float32).reshape(E, C))
    in_maps = []
    for i in range(NCORES):
        es = slice(i * EPC, (i + 1) * EPC)
        in_maps.append({
            "xs": np.ascontiguousarray(x[:, i * EPC * C:(i + 1) * EPC * C, :]),
            "w1t": np.ascontiguousarray(w1t[es]),
            "b1s": np.ascontiguousarray(b1r[es]),
            "w2t": np.ascontiguousarray(w2t[es]),
            "b2s": np.ascontiguousarray(b2r[es]),
        })
    return in_maps


def run(x, W1, b1, W2, b2, trace=False, **trace_kwargs):
    nc = _get_nc()
    in_maps = _shard_inputs(x, W1, b1, W2, b2)
    res = run_bass_kernel_spmd(
        nc, in_maps, core_ids=list(range(NCORES)), trace=trace, **trace_kwargs
    )
    y = np.concatenate([res.results[i]["ys"] for i in range(NCORES)], axis=1)
    return y, res


def kernel(x, W1, b1, W2, b2):
    y, _ = run(x, W1, b1, W2, b2)
    return y.astype(np.float32)


# revision 18
# speedup vs baseline: 1.0315x; 1.0004x over previous
"""Expert-parallel grouped-MLP (MoE experts) kernel for 8 Trainium2 cores.

Problem: y = W2_e @ silu(W1_e @ x_e + b1_e) + b2_e for E=16 independent
experts (grouped 1x1 conv), B=8 batches, C=256 channels/expert, CAP=4,
L=1024 positions.

Sharding: expert-parallel — core i owns experts {2i, 2i+1}; no cross-core
communication. Host pre-transposes weights into lhsT layout; each core runs
an fp16 matmul pipeline (DVE casts fp32->fp16 on the fly):

  per (b, e) pair:  DMA x[256,1024] -> 2 k-tiles
    layer 1: 8 m-tiles x (2k x 2n) matmuls -> PSUM[128,1024]
             ACT silu(. + b1) PSUM -> h SBUF [128, 8x1024] (fp16)
    layer 2: 2 j-tiles x (8q x 2n) accumulating matmuls -> PSUM[128,1024]
             DVE + b2 PSUM -> y SBUF, DMA out

Startup: 16 zero-weight bf16 warmup matmuls keep the PE busy (and HAM
warm) while the first weights/x stream in; DMA issue order puts pair-0
data first and defers expert-1 weights. Measured ~244-247 us HW exec,
rel err 4.3e-4 (fp16 matmuls, fp32 accumulate).
"""
import numpy as np

import concourse.tile as tile
from concourse import bacc, mybir
from concourse.bass_utils import run_bass_kernel_spmd

# Problem constants (hardcoded per contract)
B, E, C, CAP, L = 8, 16, 256, 4, 1024
F = C * CAP            # 1024 hidden per expert
NCORES = 8
EPC = E // NCORES      # 2 experts per core
P = 128                # partitions
KT = C // P            # 2 k-tiles (layer-1 contraction)
MT = F // P            # 8 m-tiles (layer-1 output partitions)
JT = C // P            # 2 j-tiles (layer-2 output partitions)
QT = F // P            # 8 q-tiles (layer-2 contraction)
NT = L // 512          # 2 n-tiles of 512 cols
N_WARMUP = 16          # dummy PE warmup matmuls

_FP32 = mybir.dt.float32
_FP32R = mybir.dt.float32r
_FP16 = mybir.dt.float16


def _build():
    nc = bacc.Bacc("TRN2", target_bir_lowering=False, debug=False)

    xs_d = nc.dram_tensor("xs", [B, EPC * C, L], _FP32, kind="ExternalInput")
    w1t_d = nc.dram_tensor("w1t", [EPC, C, F], _FP32, kind="ExternalInput")
    b1s_d = nc.dram_tensor("b1s", [EPC, F], _FP32, kind="ExternalInput")
    w2t_d = nc.dram_tensor("w2t", [EPC, F, C], _FP32, kind="ExternalInput")
    b2s_d = nc.dram_tensor("b2s", [EPC, C], _FP32, kind="ExternalInput")
    ys_d = nc.dram_tensor("ys", [B, EPC * C, L], _FP32, kind="ExternalOutput")

    with tile.TileContext(nc) as tc:
        with (
            tc.tile_pool(name="const", bufs=1) as cpool,
            tc.tile_pool(name="x", bufs=8) as xpool,
            tc.tile_pool(name="h", bufs=2) as hpool,
            tc.tile_pool(name="y", bufs=3) as ypool,
            tc.tile_pool(name="ps", bufs=4, space="PSUM") as pspool,
        ):
            # ---- PE warmup: zero bf16 matmuls with no DMA deps ----
            wdum = cpool.tile([P, P], mybir.dt.bfloat16, tag="wdum")
            rdum = cpool.tile([P, 512], mybir.dt.bfloat16, tag="rdum")
            nc.vector.memset(wdum[:], 0.0)
            nc.vector.memset(rdum[:], 0.0)
            actdum = cpool.tile([P, 1], _FP32, tag="actdum")
            nc.scalar.activation(actdum[:], rdum[:, :1],
                                 mybir.ActivationFunctionType.Silu, bias=0.0)
            for i in range(N_WARMUP):
                pdum = pspool.tile([P, L], _FP32, tag="ps")
                nc.tensor.matmul(pdum[:, :512], wdum[:], rdum[:],
                                 start=True, stop=True)

            # ---- weight/bias tiles (declared; DMAs ordered for startup) ----
            # w1sb[e][k]: [128, F];   [p, f] = W1T[e, k*128+p, f]
            # w2sb[e]:    [128, QT*C]; [p, q*C+c] = W2T[e, q*128+p, c]
            w1sb = [[cpool.tile([P, F], _FP16, tag=f"w1_{e}_{k}",
                               name=f"w1sb_{e}_{k}")
                     for k in range(KT)] for e in range(EPC)]
            w2sb = [cpool.tile([P, QT * C], _FP16, tag=f"w2_{e}",
                               name=f"w2sb_{e}")
                    for e in range(EPC)]
            w1st = [[cpool.tile([P, F], _FP32, tag=f"w1s_{e}_{k}",
                                name=f"w1st_{e}_{k}")
                     for k in range(KT)] for e in range(EPC)]
            w2st = [cpool.tile([P, QT * C], _FP32, tag=f"w2s_{e}",
                               name=f"w2st_{e}")
                    for e in range(EPC)]
            b1sb = cpool.tile([P, EPC * MT], _FP32, tag="b1")  # col e*MT+m
            b2sb = cpool.tile([P, EPC * JT], _FP32, tag="b2")  # col e*JT+j

            def load_w1(e, k):
                nc.sync.dma_start(
                    w1st[e][k][:],
                    w1t_d.ap()[e, k * P:(k + 1) * P, :],
                )
                nc.vector.tensor_copy(w1sb[e][k][:], w1st[e][k][:])

            def load_w2(e, q):
                nc.sync.dma_start(
                    w2st[e][:, q * C:(q + 1) * C],
                    w2t_d.ap()[e, q * P:(q + 1) * P, :],
                )

            def cast_w2(e):
                nc.vector.tensor_copy(w2sb[e][:], w2st[e][:])

            def load_b(e):
                nc.sync.dma_start(
                    b1sb[:, e * MT:(e + 1) * MT],
                    b1s_d.ap()[e].rearrange("(m p) -> p m", p=P),
                )
                nc.sync.dma_start(
                    b2sb[:, e * JT:(e + 1) * JT],
                    b2s_d.ap()[e].rearrange("(j p) -> p j", p=P),
                )

            def load_x(b, e):
                # two k-tiles [128, L], each a contiguous 512 KB DRAM slab,
                # staged fp32 then DVE-cast to fp16
                tiles = []
                for k in range(KT):
                    xst = xpool.tile([P, L], _FP32, tag="xst", name=f"xst_{b}_{e}_{k}")
                    nc.sync.dma_start(
                        xst[:],
                        xs_d.ap()[b, e * C + k * P: e * C + (k + 1) * P, :],
                    )
                    xt = xpool.tile([P, L], _FP16, tag="x", name=f"x_{b}_{e}_{k}")
                    nc.vector.tensor_copy(xt[:], xst[:])
                    tiles.append(xt)
                return tiles

            def load_x_split(b, e):
                # first pair: per-n half tiles so MM 0 waits on 256 KB only
                tiles = []
                for k in range(KT):
                    halves = []
                    for n in range(NT):
                        xst = xpool.tile([P, 512], _FP32, tag="xst0",
                                         name=f"xst0_{k}_{n}")
                        nc.sync.dma_start(
                            xst[:],
                            xs_d.ap()[b, e * C + k * P: e * C + (k + 1) * P,
                                      n * 512:(n + 1) * 512],
                        )
                        xt = xpool.tile([P, 512], _FP16, tag="x0",
                                        name=f"x0_{k}_{n}")
                        nc.vector.tensor_copy(xt[:], xst[:])
                        halves.append(xt)
                    tiles.append(halves)
                return tiles

            # startup-critical order: pair-0 inputs first; expert-1 weights
            # deferred until after early x prefetches (needed only at pair 8)
            load_w1(0, 0)
            x0 = load_x_split(0, 0)
            load_b(0)
            load_w1(0, 1)
            for q in range(QT):
                load_w2(0, q)
            cast_w2(0)

            # ---- per-(expert, batch) pipeline ----
            for e in range(EPC):
                for b in range(B):
                    xsb = x0 if (e == 0 and b == 0) else load_x(b, e)
                    if e == 0 and b == 1:
                        # early x prefetches are in flight; now queue
                        # expert-1 weights (needed at pair 8, ~130us in)
                        load_w1(1, 0)
                        load_w1(1, 1)
                        load_b(1)
                        for q in range(QT):
                            load_w2(1, q)
                        cast_w2(1)

                    # layer 1: h = silu(W1 @ x + b1), h[p, m*L + l]
                    hsb = hpool.tile([P, MT * L], _FP16, tag="h")
                    for m in range(MT):
                        psh = pspool.tile([P, L], _FP32, tag="ps")
                        for k in range(KT):
                            for n in range(NT):
                                rhs = (xsb[k][n][:] if (e == 0 and b == 0)
                                       else xsb[k][:, n * 512:(n + 1) * 512])
                                nc.tensor.matmul(
                                    psh[:, n * 512:(n + 1) * 512],
                                    w1sb[e][k][:, m * P:(m + 1) * P],
                                    rhs,
                                    start=(k == 0),
                                    stop=(k == KT - 1),
                                )
                        nc.scalar.activation(
                            hsb[:, m * L:(m + 1) * L],
                            psh[:],
                            mybir.ActivationFunctionType.Silu,
                            bias=b1sb[:, e * MT + m: e * MT + m + 1],
                        )

                    # layer 2: y = W2 @ h + b2
                    last_pair = (e == EPC - 1 and b == B - 1)
                    for j in range(JT):
                        psy = pspool.tile([P, L], _FP32, tag="ps")
                        if last_pair:
                            # n-outer: per-n DVE+DMA pipeline under the
                            # final matmul chains to shorten the tail
                            for n in range(NT):
                                for q in range(QT):
                                    nc.tensor.matmul(
                                        psy[:, n * 512:(n + 1) * 512],
                                        w2sb[e][:, q * C + j * P:
                                                q * C + (j + 1) * P],
                                        hsb[:, q * L + n * 512:
                                              q * L + (n + 1) * 512],
                                        start=(q == 0),
                                        stop=(q == QT - 1),
                                    )
                                ysn = ypool.tile([P, 512], _FP32, tag="y",
                                                 name=f"ysn_{j}_{n}")
                                nc.vector.tensor_scalar_add(
                                    ysn[:],
                                    psy[:, n * 512:(n + 1) * 512],
                                    b2sb[:, e * JT + j: e * JT + j + 1],
                                )
                                nc.sync.dma_start(
                                    ys_d.ap()[b,
                                              e * C + j * P: e * C + (j + 1) * P,
                                              n * 512:(n + 1) * 512],
                                    ysn[:],
                                )
                            continue
                        for q in range(QT):
                            for n in range(NT):
                                nc.tensor.matmul(
                                    psy[:, n * 512:(n + 1) * 512],
                                    w2sb[e][:, q * C + j * P: q * C + (j + 1) * P],
                                    hsb[:, q * L + n * 512: q * L + (n + 1) * 512],
                                    start=(q == 0),
                                    stop=(q == QT - 1),
                                )
                        ysb = ypool.tile([P, L], _FP32, tag="y",
                                         name=f"ysb_{e}_{b}_{j}")
                        nc.vector.tensor_scalar_add(
                            ysb[:],
                            psy[:],
                            b2sb[:, e * JT + j: e * JT + j + 1],
                        )
                        nc.sync.dma_start(
                            ys_d.ap()[b, e * C + j * P: e * C + (j + 1) * P, :],
                            ysb[:],
                        )

    nc.compile()
    return nc


_NC_CACHE = None


def _get_nc():
    global _NC_CACHE
    if _NC_CACHE is None:
        _NC_CACHE = _build()
    return _NC_CACHE


def _shard_inputs(x, W1, b1, W2, b2):
    """Full inputs -> list of 8 per-core input dicts (expert-parallel)."""
    x = np.ascontiguousarray(x, dtype=np.float32)
    # lhsT layouts: W1T[e] = W1r[e].T -> [E, C, F]; W2T[e] = W2r[e].T -> [E, F, C]
    w1t = np.ascontiguousarray(
        W1.astype(np.float32).reshape(E, F, C).transpose(0, 2, 1)
    )
    w2t = np.ascontiguousarray(
        W2.astype(np.float32).reshape(E, C, F).transpose(0, 2, 1)
    )
    b1r = np.ascontiguousarray(b1.astype(np.float32).reshape(E, F))
    b2r = np.ascontiguousarray(b2.astype(np.float32).reshape(E, C))
    in_maps = []
    for i in range(NCORES):
        es = slice(i * EPC, (i + 1) * EPC)
        in_maps.append({
            "xs": np.ascontiguousarray(x[:, i * EPC * C:(i + 1) * EPC * C, :]),
            "w1t": np.ascontiguousarray(w1t[es]),
            "b1s": np.ascontiguousarray(b1r[es]),
            "w2t": np.ascontiguousarray(w2t[es]),
            "b2s": np.ascontiguousarray(b2r[es]),
        })
    return in_maps


def run(x, W1, b1, W2, b2, trace=False, **trace_kwargs):
    nc = _get_nc()
    in_maps = _shard_inputs(x, W1, b1, W2, b2)
    res = run_bass_kernel_spmd(
        nc, in_maps, core_ids=list(range(NCORES)), trace=trace, **trace_kwargs
    )
    y = np.concatenate([res.results[i]["ys"] for i in range(NCORES)], axis=1)
    return y, res


def kernel(x, W1, b1, W2, b2):
    y, _ = run(x, W1, b1, W2, b2)
    return y.astype(np.float32)


# revision 19
# speedup vs baseline: 1.0336x; 1.0020x over previous
"""Expert-parallel grouped-MLP (MoE experts) kernel for 8 Trainium2 cores.

Problem: y = W2_e @ silu(W1_e @ x_e + b1_e) + b2_e for E=16 independent
experts (grouped 1x1 conv), B=8 batches, C=256 channels/expert, CAP=4,
L=1024 positions.

Sharding: expert-parallel — core i owns experts {2i, 2i+1}; no cross-core
communication. Host pre-transposes weights into lhsT layout; each core runs
an fp16 matmul pipeline (DVE casts fp32->fp16 on the fly):

  per (b, e) pair:  DMA x[256,1024] -> 2 k-tiles
    layer 1: 8 m-tiles x (2k x 2n) matmuls -> PSUM[128,1024]
             ACT silu(. + b1) PSUM -> h SBUF [128, 8x1024] (fp16)
    layer 2: 2 j-tiles x (8q x 2n) accumulating matmuls -> PSUM[128,1024]
             DVE + b2 PSUM -> y SBUF, DMA out

Startup: 16 zero-weight bf16 warmup matmuls keep the PE busy (and HAM
warm) while the first weights/x stream in; DMA issue order puts pair-0
data first and defers expert-1 weights. Measured ~244-247 us HW exec,
rel err 4.3e-4 (fp16 matmuls, fp32 accumulate).
"""
import numpy as np

import concourse.tile as tile
from concourse import bacc, mybir
from concourse.bass_utils import run_bass_kernel_spmd

# Problem constants (hardcoded per contract)
B, E, C, CAP, L = 8, 16, 256, 4, 1024
F = C * CAP            # 1024 hidden per expert
NCORES = 8
EPC = E // NCORES      # 2 experts per core
P = 128                # partitions
KT = C // P            # 2 k-tiles (layer-1 contraction)
MT = F // P            # 8 m-tiles (layer-1 output partitions)
JT = C // P            # 2 j-tiles (layer-2 output partitions)
QT = F // P            # 8 q-tiles (layer-2 contraction)
NT = L // 512          # 2 n-tiles of 512 cols
N_WARMUP = 16          # dummy PE warmup matmuls

_FP32 = mybir.dt.float32
_FP32R = mybir.dt.float32r
_FP16 = mybir.dt.float16


def _build():
    nc = bacc.Bacc("TRN2", target_bir_lowering=False, debug=False)

    xs_d = nc.dram_tensor("xs", [B, EPC * C, L], _FP32, kind="ExternalInput")
    w1t_d = nc.dram_tensor("w1t", [EPC, C, F], _FP32, kind="ExternalInput")
    b1s_d = nc.dram_tensor("b1s", [EPC, F], _FP32, kind="ExternalInput")
    w2t_d = nc.dram_tensor("w2t", [EPC, F, C], _FP32, kind="ExternalInput")
    b2s_d = nc.dram_tensor("b2s", [EPC, C], _FP32, kind="ExternalInput")
    ys_d = nc.dram_tensor("ys", [B, EPC * C, L], _FP32, kind="ExternalOutput")

    with tile.TileContext(nc) as tc:
        with (
            tc.tile_pool(name="const", bufs=1) as cpool,
            tc.tile_pool(name="x", bufs=8) as xpool,
            tc.tile_pool(name="h", bufs=2) as hpool,
            tc.tile_pool(name="y", bufs=3) as ypool,
            tc.tile_pool(name="ps", bufs=4, space="PSUM") as pspool,
        ):
            # ---- PE warmup: zero bf16 matmuls with no DMA deps ----
            wdum = cpool.tile([P, P], mybir.dt.bfloat16, tag="wdum")
            rdum = cpool.tile([P, 512], mybir.dt.bfloat16, tag="rdum")
            nc.vector.memset(wdum[:], 0.0)
            nc.vector.memset(rdum[:], 0.0)
            for i in range(N_WARMUP):
                pdum = pspool.tile([P, L], _FP32, tag="ps")
                nc.tensor.matmul(pdum[:, :512], wdum[:], rdum[:],
                                 start=True, stop=True)

            # ---- weight/bias tiles (declared; DMAs ordered for startup) ----
            # w1sb[e][k]: [128, F];   [p, f] = W1T[e, k*128+p, f]
            # w2sb[e]:    [128, QT*C]; [p, q*C+c] = W2T[e, q*128+p, c]
            w1sb = [[cpool.tile([P, F], _FP16, tag=f"w1_{e}_{k}",
                               name=f"w1sb_{e}_{k}")
                     for k in range(KT)] for e in range(EPC)]
            w2sb = [cpool.tile([P, QT * C], _FP16, tag=f"w2_{e}",
                               name=f"w2sb_{e}")
                    for e in range(EPC)]
            w1st = [[cpool.tile([P, F], _FP32, tag=f"w1s_{e}_{k}",
                                name=f"w1st_{e}_{k}")
                     for k in range(KT)] for e in range(EPC)]
            w2st = [cpool.tile([P, QT * C], _FP32, tag=f"w2s_{e}",
                               name=f"w2st_{e}")
                    for e in range(EPC)]
            b1sb = cpool.tile([P, EPC * MT], _FP32, tag="b1")  # col e*MT+m
            b2sb = cpool.tile([P, EPC * JT], _FP32, tag="b2")  # col e*JT+j

            def load_w1(e, k):
                nc.sync.dma_start(
                    w1st[e][k][:],
                    w1t_d.ap()[e, k * P:(k + 1) * P, :],
                )
                nc.vector.tensor_copy(w1sb[e][k][:], w1st[e][k][:])

            def load_w2(e, q):
                nc.sync.dma_start(
                    w2st[e][:, q * C:(q + 1) * C],
                    w2t_d.ap()[e, q * P:(q + 1) * P, :],
                )

            def cast_w2(e):
                nc.vector.tensor_copy(w2sb[e][:], w2st[e][:])

            def load_b(e):
                nc.sync.dma_start(
                    b1sb[:, e * MT:(e + 1) * MT],
                    b1s_d.ap()[e].rearrange("(m p) -> p m", p=P),
                )
                nc.sync.dma_start(
                    b2sb[:, e * JT:(e + 1) * JT],
                    b2s_d.ap()[e].rearrange("(j p) -> p j", p=P),
                )

            def load_x(b, e):
                # two k-tiles [128, L], each a contiguous 512 KB DRAM slab,
                # staged fp32 then DVE-cast to fp16
                tiles = []
                for k in range(KT):
                    xst = xpool.tile([P, L], _FP32, tag="xst", name=f"xst_{b}_{e}_{k}")
                    nc.sync.dma_start(
                        xst[:],
                        xs_d.ap()[b, e * C + k * P: e * C + (k + 1) * P, :],
                    )
                    xt = xpool.tile([P, L], _FP16, tag="x", name=f"x_{b}_{e}_{k}")
                    nc.vector.tensor_copy(xt[:], xst[:])
                    tiles.append(xt)
                return tiles

            def load_x_split(b, e):
                # first pair: per-n half tiles so MM 0 waits on 256 KB only
                tiles = []
                for k in range(KT):
                    halves = []
                    for n in range(NT):
                        xst = xpool.tile([P, 512], _FP32, tag="xst0",
                                         name=f"xst0_{k}_{n}")
                        nc.sync.dma_start(
                            xst[:],
                            xs_d.ap()[b, e * C + k * P: e * C + (k + 1) * P,
                                      n * 512:(n + 1) * 512],
                        )
                        xt = xpool.tile([P, 512], _FP16, tag="x0",
                                        name=f"x0_{k}_{n}")
                        nc.vector.tensor_copy(xt[:], xst[:])
                        halves.append(xt)
                    tiles.append(halves)
                return tiles

            # startup-critical order: pair-0 inputs first; expert-1 weights
            # deferred until after early x prefetches (needed only at pair 8)
            load_w1(0, 0)
            x0 = load_x_split(0, 0)
            load_w1(0, 1)
            load_b(0)
            for q in range(QT):
                load_w2(0, q)
            cast_w2(0)

            # ---- per-(expert, batch) pipeline ----
            for e in range(EPC):
                for b in range(B):
                    xsb = x0 if (e == 0 and b == 0) else load_x(b, e)
                    if e == 0 and b == 1:
                        # early x prefetches are in flight; now queue
                        # expert-1 weights (needed at pair 8, ~130us in)
                        load_w1(1, 0)
                        load_w1(1, 1)
                        load_b(1)
                        for q in range(QT):
                            load_w2(1, q)
                        cast_w2(1)

                    # layer 1: h = silu(W1 @ x + b1), h[p, m*L + l]
                    hsb = hpool.tile([P, MT * L], _FP16, tag="h")
                    for m in range(MT):
                        psh = pspool.tile([P, L], _FP32, tag="ps")
                        for k in range(KT):
                            for n in range(NT):
                                rhs = (xsb[k][n][:] if (e == 0 and b == 0)
                                       else xsb[k][:, n * 512:(n + 1) * 512])
                                nc.tensor.matmul(
                                    psh[:, n * 512:(n + 1) * 512],
                                    w1sb[e][k][:, m * P:(m + 1) * P],
                                    rhs,
                                    start=(k == 0),
                                    stop=(k == KT - 1),
                                )
                        nc.scalar.activation(
                            hsb[:, m * L:(m + 1) * L],
                            psh[:],
                            mybir.ActivationFunctionType.Silu,
                            bias=b1sb[:, e * MT + m: e * MT + m + 1],
                        )

                    # layer 2: y = W2 @ h + b2
                    for j in range(JT):
                        psy = pspool.tile([P, L], _FP32, tag="ps")
                        for q in range(QT):
                            for n in range(NT):
                                nc.tensor.matmul(
                                    psy[:, n * 512:(n + 1) * 512],
                                    w2sb[e][:, q * C + j * P: q * C + (j + 1) * P],
                                    hsb[:, q * L + n * 512: q * L + (n + 1) * 512],
                                    start=(q == 0),
                                    stop=(q == QT - 1),
                                )
                        ysb = ypool.tile([P, L], _FP32, tag="y",
                                         name=f"ysb_{e}_{b}_{j}")
                        nc.vector.tensor_scalar_add(
                            ysb[:],
                            psy[:],
                            b2sb[:, e * JT + j: e * JT + j + 1],
                        )
                        nc.sync.dma_start(
                            ys_d.ap()[b, e * C + j * P: e * C + (j + 1) * P, :],
                            ysb[:],
                        )

    nc.compile()
    return nc


_NC_CACHE = None


def _get_nc():
    global _NC_CACHE
    if _NC_CACHE is None:
        _NC_CACHE = _build()
    return _NC_CACHE


def _shard_inputs(x, W1, b1, W2, b2):
    """Full inputs -> list of 8 per-core input dicts (expert-parallel)."""
    x = np.ascontiguousarray(x, dtype=np.float32)
    # lhsT layouts: W1T[e] = W1r[e].T -> [E, C, F]; W2T[e] = W2r[e].T -> [E, F, C]
    w1t = np.ascontiguousarray(
        W1.astype(np.float32).reshape(E, F, C).transpose(0, 2, 1)
    )
    w2t = np.ascontiguousarray(
        W2.astype(np.float32).reshape(E, C, F).transpose(0, 2, 1)
    )
    b1r = np.ascontiguousarray(b1.astype(np.float32).reshape(E, F))
    b2r = np.ascontiguousarray(b2.astype(np.float32).reshape(E, C))
    in_maps = []
    for i in range(NCORES):
        es = slice(i * EPC, (i + 1) * EPC)
        in_maps.append({
            "xs": np.ascontiguousarray(x[:, i * EPC * C:(i + 1) * EPC * C, :]),
            "w1t": np.ascontiguousarray(w1t[es]),
            "b1s": np.ascontiguousarray(b1r[es]),
            "w2t": np.ascontiguousarray(w2t[es]),
            "b2s": np.ascontiguousarray(b2r[es]),
        })
    return in_maps


def run(x, W1, b1, W2, b2, trace=False, **trace_kwargs):
    nc = _get_nc()
    in_maps = _shard_inputs(x, W1, b1, W2, b2)
    res = run_bass_kernel_spmd(
        nc, in_maps, core_ids=list(range(NCORES)), trace=trace, **trace_kwargs
    )
    y = np.concatenate([res.results[i]["ys"] for i in range(NCORES)], axis=1)
    return y, res


def kernel(x, W1, b1, W2, b2):
    y, _ = run(x, W1, b1, W2, b2)
    return y.astype(np.float32)
